# revision 1
# baseline (speedup 1.0000x reference)
"""Trainium2 Bass kernel for nn_Discriminator: LSTM-last-h + 2 causal convs + dense head.

Data-parallel over 8 NeuronCores (batch 1024 -> 128 per core).

Design (per core, batch Bc=128):
  - "Feature-major" (transposed) layout throughout: on-chip tensors are
    [channel, batch] so the LSTM gate epilogue directly produces h^T, the
    streaming operand of the next step's recurrent matmuls (no per-step
    transposes).
  - x is transposed once on entry via PE-transpose into x2T [103, 256, 128]
    (bf16): rows 0:51 = features of even t, 51:102 = odd t, row 102 = ones
    (used to fold the LSTM bias into the input projection).
  - LSTM scan: per step, z^T (8 tiles of [128,128] fp32 in PSUM) accumulates
    the input projection (emitted 2 steps ahead; K=103 vs zero-padded
    even/odd Wx stationaries) and the recurrence (16 matmuls, Wh blocks
    stationary, h^T streaming). Gate order is host-permuted to [f,i,o,g] so
    sigmoid(f,i) is a single [128,512] ACT op.
  - Convs: stride-2 causal convs become 3 accumulating matmuls per output
    tile (tap pairs packed along K using the even/odd partition packing),
    weights stationary; LeakyReLU+bias via one ACT op.
  - Dense head: per-to2 matvec matmuls (M=1) accumulating into PSUM.
"""

import os
import sys

# Reset cores on session open: stale device state from a previous run
# (crashed or otherwise) can silently corrupt results without this.
os.environ.setdefault("NEURON_RT_RESET_CORES", "1")

sys.path.insert(0, "/opt/trn_rl_repo")

import numpy as np
import ml_dtypes
from contextlib import ExitStack

import concourse.bass as bass
import concourse.tile as tile
from concourse import bacc, mybir
from concourse.bass_utils import run_bass_kernel_spmd

F32 = mybir.dt.float32
BF16 = mybir.dt.bfloat16
AF = mybir.ActivationFunctionType

B, T, F, H = 1024, 512, 51, 256
NCORES = 8
BC = B // NCORES  # 128
T2 = T // 2  # 256
ALPHA = 0.3

# dtype knobs
WH_BF16 = True   # recurrent weights + h in bf16 (FWL weight loads)
X_BF16 = True    # x / input-projection path in bf16

_NC_CACHE = {}
DEBUG_DUMP = False  # add aT/h1T/x2T debug outputs
DEBUG_Z_STEP = 0


def _dt(np_arr, bf16):
    return np_arr.astype(ml_dtypes.bfloat16) if bf16 else np_arr.astype(np.float32)


def build_nc(t_steps=T, reps=1):
    """Build + compile the single-core SPMD program.

    reps > 1 repeats the whole computation (for differential timing).
    """
    dt_x = BF16 if X_BF16 else F32
    dt_wh = BF16 if WH_BF16 else F32
    nt2 = (t_steps + 1) // 2

    nc = bacc.Bacc("TRN2", target_bir_lowering=False, debug=False)

    x2t_d = nc.dram_tensor("x2t", [103, (t_steps + 1) // 2, BC], dt_x,
                           kind="ExternalInput").ap()
    wxe_d = nc.dram_tensor("wxe", [103, 8, 128], dt_x, kind="ExternalInput").ap()
    wxo_d = nc.dram_tensor("wxo", [103, 8, 128], dt_x, kind="ExternalInput").ap()
    whT_d = nc.dram_tensor("whT", [128, 2, 8, 128], dt_wh, kind="ExternalInput").ap()
    k1p_d = nc.dram_tensor("k1p", [102, 3, 64], BF16, kind="ExternalInput").ap()
    k2p_d = nc.dram_tensor("k2p", [128, 3, 128], BF16, kind="ExternalInput").ap()
    wdb_d = nc.dram_tensor("wdb", [128, 128], F32, kind="ExternalInput").ap()
    wda_d = nc.dram_tensor("wda", [128, 2], F32, kind="ExternalInput").ap()
    b1_d = nc.dram_tensor("b1", [64, 1], F32, kind="ExternalInput").ap()
    b2_d = nc.dram_tensor("b2", [128, 1], F32, kind="ExternalInput").ap()
    bd_d = nc.dram_tensor("bd", [1, 1], F32, kind="ExternalInput").ap()
    y_d = nc.dram_tensor("y", [1, BC], F32, kind="ExternalOutput").ap()
    if DEBUG_DUMP:
        nt2_dbg = (t_steps + 1) // 2
        dbg_x2T = nc.dram_tensor("dbg_x2T", [103, nt2_dbg, BC], dt_x,
                                 kind="ExternalOutput").ap()
        dbg_aT = nc.dram_tensor("dbg_aT", [128, 256], F32,
                                kind="ExternalOutput").ap()
        dbg_h1T = nc.dram_tensor("dbg_h1T", [128, t_steps // 4, BC], BF16,
                                 kind="ExternalOutput").ap()
        dbg_z0 = nc.dram_tensor("dbg_z0", [128, 1024], F32,
                                kind="ExternalOutput").ap()

    n_to1 = t_steps // 2      # conv1 output length (256 at full size)
    n_t4 = n_to1 // 2         # h1T pair dim
    n_to2 = n_to1 // 2        # conv2 output length

    with TileCtx(nc) as tc, ExitStack() as ctx:
        singles = ctx.enter_context(tc.tile_pool(name="singles", bufs=1))

        x2T = singles.tile([103, nt2, BC], dt_x)
        wxe = singles.tile([103, 8, 128], dt_x)
        wxo = singles.tile([103, 8, 128], dt_x)
        whT = singles.tile([128, 2, 8, 128], dt_wh)
        k1p = singles.tile([102, 3, 64], BF16)
        k2p = singles.tile([128, 3, 128], BF16)
        wdb = singles.tile([128, 128], F32)
        wda = singles.tile([128, 2], F32)
        b1s = singles.tile([64, 1], F32)
        b2s = singles.tile([128, 1], F32)
        bds = singles.tile([1, 1], F32)
        aT = singles.tile([128, 256], F32)
        zst = singles.tile([128, 128], BF16)
        nc.vector.memset(zst[:], 0.0)
        c1tmp = singles.tile([64, 2, BC], BF16)

        # scan-critical tensors first, spread across engine DMA queues so
        # they load in parallel: the first LSTM step only needs the
        # recurrence/projection weights and the first x2T slice; the small
        # conv/head weights next (conv1 chunk 0 runs at step 0 and the
        # in-order PE queue would stall on them); bulk x2T slices last.
        nc.sync.dma_start(whT[:], whT_d)
        nc.scalar.dma_start(wxe[:], wxe_d)
        nc.scalar.dma_start(wxo[:], wxo_d)
        # x2T comes pre-transposed (and bias-row-augmented) from the host;
        # DMA it in slices so the scan can start before the load completes.
        NSL = 16
        sl = (nt2 + NSL - 1) // NSL
        nc.gpsimd.dma_start(x2T[:, 0:sl, :], x2t_d[:, 0:sl, :])
        nc.gpsimd.dma_start(k1p[:], k1p_d)
        nc.gpsimd.dma_start(b1s[:], b1_d)
        nc.sync.dma_start(k2p[:], k2p_d)
        nc.sync.dma_start(b2s[:], b2_d)
        nc.sync.dma_start(wdb[:], wdb_d)
        nc.sync.dma_start(wda[:], wda_d)
        nc.sync.dma_start(bds[:], bd_d)
        for s in range(1, NSL):
            s0, s1 = s * sl, min((s + 1) * sl, nt2)
            if s0 < s1:
                nc.gpsimd.dma_start(x2T[:, s0:s1, :], x2t_d[:, s0:s1, :])

        h1T = singles.tile([128, n_t4, BC], BF16)
        # holder for the current step's tanh_c tile (set by scan_step); used
        # by the conv helpers to gate their activations behind the c-chain
        tc_hold = [None]

        def emit_conv1_chunk(q, c1ps):
            p1 = c1ps.tile([64, 4, BC], F32, tag="p1")
            if q == 0:
                # causal left edge, per-to1; one start=True for the bank
                first = True
                for i, to1 in enumerate((0, 1, 2, 3)):
                    dst = p1[:, i, :]
                    mms = []
                    if to1 >= 2:
                        mms.append((k1p[:, 0, :], x2T[0:102, to1 - 2, :]))
                    if to1 >= 1:
                        mms.append((k1p[:, 1, :], x2T[0:102, to1 - 1, :]))
                    mms.append((k1p[0:51, 2, :], x2T[0:51, to1, :]))
                    for m, (lhsT, rhs) in enumerate(mms):
                        nc.tensor.matmul(
                            dst, lhsT, rhs,
                            start=first, stop=(m == len(mms) - 1),
                            skip_group_check=True,
                        )
                        first = False
            else:
                base = 4 * q
                nc.tensor.matmul(
                    p1[:], k1p[:, 0, :], x2T[0:102, base - 2 : base + 2, :],
                    start=True, stop=False,
                )
                nc.tensor.matmul(
                    p1[:], k1p[:, 1, :], x2T[0:102, base - 1 : base + 3, :],
                    start=False, stop=False,
                )
                nc.tensor.matmul(
                    p1[:], k1p[0:51, 2, :], x2T[0:51, base : base + 4, :],
                    start=False, stop=True,
                )
            # Accumulate-zero matmul gated on this step's tanh_c: a real
            # data dependency that keeps the (otherwise-ready) Prelus below
            # from greedily preempting tanh_c on the in-order ACT queue.
            if tc_hold[0] is not None:
                nc.tensor.matmul(
                    p1[:, 0:2, :], zst[:, 0:64], tc_hold[0][:],
                    start=False, stop=True, skip_group_check=True,
                )
            # LeakyReLU + bias; scatter even/odd to1 to partition halves.
            # Even half on ACT; odd half on the (idle) DVE so the conv-step
            # ACT tail doesn't collide with the next step's sigmoid:
            # leaky(x+b) = max(x+b, 0.3*(x+b))
            nc.scalar.activation(
                h1T[0:64, 2 * q : 2 * q + 2, :], p1[:, 0::2, :],
                AF.Prelu, bias=b1s[:], alpha=ALPHA,
            )
            nc.vector.tensor_scalar_add(c1tmp[:], p1[:, 1::2, :], b1s[:])
            nc.vector.scalar_tensor_tensor(
                h1T[64:128, 2 * q : 2 * q + 2, :], c1tmp[:], ALPHA,
                c1tmp[:], mybir.AluOpType.mult, mybir.AluOpType.max,
            )

        def emit_conv2_chunk(q, c2ps, h2sb, mvps, acc):
            p2 = c2ps.tile([128, 4, BC], F32, tag="p2")
            if q == 0:
                first = True
                for i, to2 in enumerate((0, 1, 2, 3)):
                    dst = p2[:, i, :]
                    mms = []
                    if to2 >= 2:
                        mms.append((k2p[:, 0, :], h1T[:, to2 - 2, :]))
                    if to2 >= 1:
                        mms.append((k2p[:, 1, :], h1T[:, to2 - 1, :]))
                    mms.append((k2p[0:64, 2, :], h1T[0:64, to2, :]))
                    for m, (lhsT, rhs) in enumerate(mms):
                        nc.tensor.matmul(
                            dst, lhsT, rhs,
                            start=first, stop=(m == len(mms) - 1),
                            skip_group_check=True,
                        )
                        first = False
            else:
                base = 4 * q
                nc.tensor.matmul(
                    p2[:], k2p[:, 0, :], h1T[:, base - 2 : base + 2, :],
                    start=True, stop=False,
                )
                nc.tensor.matmul(
                    p2[:], k2p[:, 1, :], h1T[:, base - 1 : base + 3, :],
                    start=False, stop=False,
                )
                nc.tensor.matmul(
                    p2[:], k2p[0:64, 2, :], h1T[0:64, base : base + 4, :],
                    start=False, stop=True,
                )
            # accumulate-zero matmul gated on this step's tanh_c: keeps the
            # (otherwise-ready) h2 Prelu from greedily preempting tanh_c
            if tc_hold[0] is not None:
                nc.tensor.matmul(
                    p2[:, 0:2, :], zst[:], tc_hold[0][:],
                    start=False, stop=True, skip_group_check=True,
                )
            return p2

        def emit_conv2_act(p2, h2sb):
            # h2 activation, deferred to the NEXT step's ACT head-idle so it
            # cannot slot ahead of the critical-path tanh_c
            h2 = h2sb.tile([128, 4, BC], F32, tag="h2")
            nc.scalar.activation(
                h2[:], p2[:], AF.Prelu, bias=b2s[:], alpha=ALPHA
            )
            return h2

        def emit_mv(q, h2, mvps, acc):
            # head matvec for conv2 chunk q; deferred one step so the PE
            # queue doesn't stall on the late h2 activation
            mv = mvps.tile([1, BC], F32, tag="mv")
            for i in range(4):
                nc.tensor.matmul(
                    mv[:], wdb[:, 4 * q + i : 4 * q + i + 1], h2[:, i, :],
                    start=(i == 0), stop=(i == 3),
                )
            nc.vector.tensor_add(acc[:], acc[:], mv[:])

        for rep in range(reps):
            # ---- Phase B: LSTM scan with convs/head interleaved ----
            # z is split into two single-bank PSUM tiles so the epilogue's
            # sigmoid(f,i) only waits on the 8 f/i recurrence matmuls (not
            # all 16) and the t+2 input projections unblock early, keeping
            # the PE busy (HAM stays at full clock).
            with tc.tile_pool(name="zfip", bufs=3, space="PSUM") as zfip, \
                 tc.tile_pool(name="zgop", bufs=3, space="PSUM") as zgop, \
                 tc.tile_pool(name="ep", bufs=3) as ep, \
                 tc.tile_pool(name="state", bufs=3) as st:
                hT = st.tile([128, 256], dt_wh, tag="hT")
                c_st = st.tile([128, 256], BF16, tag="c")
                nc.vector.memset(hT[:], 0.0)
                nc.vector.memset(c_st[:], 0.0)

                zqf = []
                zqg = []

                def emit_xz(t):
                    zfi = zfip.tile([128, 512], F32, tag="zfi")
                    zgo = zgop.tile([128, 512], F32, tag="zgo")
                    zqf.append(zfi)
                    zqg.append(zgo)
                    wx = wxe if t % 2 == 0 else wxo
                    rhs = x2T[:, t // 2, :]
                    # start=True clears the target PSUM bank's has_written;
                    # only the first matmul into each bank gets it.
                    for j in range(4):
                        nc.tensor.matmul(
                            zfi[:, j * 128 : (j + 1) * 128],
                            wx[:, j, :],
                            rhs,
                            start=(j == 0),
                            stop=False,
                            skip_group_check=True,
                        )
                    for j in range(4, 8):
                        nc.tensor.matmul(
                            zgo[:, (j - 4) * 128 : (j - 3) * 128],
                            wx[:, j, :],
                            rhs,
                            start=(j == 4),
                            stop=False,
                            skip_group_check=True,
                        )

                def scan_step(t, filler=None, filler_post=None, pre=None):
                    nonlocal hT, c_st, last_sfi, prev_hT, last_tc
                    if pre is not None:
                        pre()
                    prev_hT = hT
                    zfi = zqf.pop(0)
                    zgo = zqg.pop(0)
                    # recurrence; f,i first (unblocks sigmoid after 8 MMs),
                    # then g, then o (only needed late)
                    for j in (0, 1, 2, 3, 6, 7, 4, 5):
                        dst = zfi if j < 4 else zgo
                        col = j * 128 if j < 4 else (j - 4) * 128
                        for cc in range(2):
                            nc.tensor.matmul(
                                dst[:, col : col + 128],
                                whT[:, cc, j, :],
                                hT[:, cc * 128 : (cc + 1) * 128],
                                start=False,
                                stop=(cc == 1),
                                skip_group_check=True,
                            )
                    if DEBUG_DUMP and t == DEBUG_Z_STEP:
                        zdump = ep.tile([128, 1024], F32, tag="zdump")
                        nc.vector.tensor_copy(zdump[:, 0:512], zfi[:])
                        nc.vector.tensor_copy(zdump[:, 512:1024], zgo[:])
                        nc.sync.dma_start(dbg_z0, zdump[:])
                    t_g = ep.tile([128, 256], BF16, tag="tg")
                    nc.scalar.activation(t_g[:], zgo[:, 256:512], AF.Tanh)
                    s_fi = ep.tile([128, 512], BF16, tag="sfi")
                    nc.scalar.activation(s_fi[:], zfi[:], AF.Sigmoid)
                    last_sfi = s_fi
                    s_o = ep.tile([128, 256], BF16, tag="so")
                    nc.scalar.activation(s_o[:], zgo[:, 0:256], AF.Sigmoid)
                    # fc first: its input (s_f) is ready before t_g, so it
                    # hides under tanh_g on the in-order DVE queue
                    fc = ep.tile([128, 256], BF16, tag="fc")
                    nc.vector.tensor_mul(fc[:], s_fi[:, 0:256], c_st[:])
                    ig = ep.tile([128, 256], BF16, tag="ig")
                    nc.vector.tensor_mul(ig[:], s_fi[:, 256:512], t_g[:])
                    c_new = st.tile([128, 256], BF16, tag="c")
                    nc.vector.tensor_add(c_new[:], fc[:], ig[:])
                    tc_t = ep.tile([128, 256], BF16, tag="tc")
                    nc.scalar.activation(tc_t[:], c_new[:], AF.Tanh)
                    last_tc = tc_t
                    tc_hold[0] = tc_t
                    h_new = st.tile([128, 256], dt_wh, tag="hT")
                    nc.vector.tensor_mul(h_new[:], s_o[:], tc_t[:])
                    hT, c_st = h_new, c_new
                    # fill the PE-idle epilogue window: conv/heater first (no
                    # waits), then the t+2 input projections (their bank
                    # clears wait on this step's PSUM readers), so the PE
                    # queue drains before the next step's h-gated matmuls
                    if filler is not None:
                        filler()
                    if t + 2 < t_steps:
                        emit_xz(t + 2)
                    if filler_post is not None:
                        filler_post()

                last_sfi = None
                prev_hT = None
                last_tc = None

                def emit_heater(pool, tag, n, rhs=None):
                    # Dummy matmuls into a conv-pool PSUM tile. They execute
                    # during the epilogue's PE-idle windows so the HAM
                    # activity monitor keeps the PE clock at 2.4 GHz (idle
                    # windows re-throttle it to 1.2 GHz, doubling the
                    # recurrence-matmul time). rhs choice sets WHEN a heater
                    # can run (its data dependency pins it to this step --
                    # otherwise the scheduler hoists all heaters early and
                    # the tail of the scan runs cold).
                    ht = pool.tile([128, 4, BC], F32, tag=tag, name="heat")
                    if rhs is not None:
                        for i in range(n):
                            nc.tensor.matmul(
                                ht[:, 0 : rhs.shape[-1] // 128, :],
                                whT[:, 0, i % 8, :], rhs,
                                start=(i == 0), stop=(i == n - 1),
                                skip_group_check=True,
                            )
                    else:
                        for i in range(n):
                            nc.tensor.matmul(
                                ht[:], wxe[:, 2 * (i % 4), :],
                                x2T[:, 4 * (i % 4) : 4 * (i % 4) + 4, :],
                                start=(i == 0), stop=(i == n - 1),
                                skip_group_check=True,
                            )

                emit_xz(0)
                if t_steps > 1:
                    emit_xz(1)
                half = t_steps // 2
                n_c1 = n_to1 // 4
                n_c2 = n_to2 // 4
                with tc.tile_pool(name="c1ps", bufs=2, space="PSUM") as c1ps:
                    if rep == 0:
                        # pre-scan warmup burst: ~20 matmuls during the DMA
                        # wait get the HAM activity window busy so the scan
                        # starts at 2.4 GHz instead of warming up mid-run
                        emit_heater(c1ps, "p1", 20)
                    for t in range(half):
                        conv_step = t % 4 == 0 and t // 4 < n_c1

                        def fill1(t=t, conv_step=conv_step):
                            # pre: right after the rec MMs (gated on h(t-1));
                            # mid: during the epilogue (gated on s_fi);
                            # end: just before the next step (gated on tanh_c)
                            emit_heater(c1ps, "p1", 4, prev_hT[:])
                            if conv_step:
                                emit_conv1_chunk(t // 4, c1ps)
                            else:
                                n_mid = 5 if t < 96 else 3
                                emit_heater(c1ps, "p1", n_mid, last_sfi[:])
                        # no tail heater on conv steps: the conv Prelu can push
                        # tanh_c late, and a tanh_c-gated heater then blocks the
                        # next step's rec matmuls in the in-order PE queue
                        scan_step(
                            t, fill1,
                            None if conv_step else
                            (lambda: emit_heater(c1ps, "p1", 2, last_tc[:])),
                        )
                    # leftover conv1 chunks (shouldn't happen at full size)
                    for q in range((half + 3) // 4, n_c1):
                        emit_conv1_chunk(q, c1ps)
                with tc.tile_pool(name="c2ps", bufs=1, space="PSUM") as c2ps, \
                     tc.tile_pool(name="h2sb", bufs=2) as h2sb, \
                     tc.tile_pool(name="mvps", bufs=1, space="PSUM") as mvps, \
                     tc.tile_pool(name="accp", bufs=1) as accp:
                    acc = accp.tile([1, BC], F32)
                    nc.vector.memset(acc[:], 0.0)
                    pending_act = []
                    pending_mv = []

                    def pre2():
                        while pending_act:
                            q, p2 = pending_act.pop(0)
                            h2 = emit_conv2_act(p2, h2sb)
                            pending_mv.append((q, h2))

                    for t in range(half, t_steps):
                        td = t - half
                        conv_step = td % 8 == 0 and td // 8 < n_c2

                        def fill2(t=t, td=td, conv_step=conv_step):
                            emit_heater(c2ps, "p2", 4, prev_hT[:])
                            if pending_mv:
                                emit_mv(*pending_mv.pop(0), mvps, acc)
                            if conv_step:
                                q = td // 8
                                p2 = emit_conv2_chunk(q, c2ps, h2sb, mvps, acc)
                                pending_act.append((q, p2))
                            else:
                                emit_heater(c2ps, "p2", 3, last_sfi[:])
                        scan_step(
                            t, fill2,
                            None if conv_step else
                            (lambda: emit_heater(c2ps, "p2", 2, last_tc[:])),
                            pre2,
                        )
                    for q in range((t_steps - half + 7) // 8, n_c2):
                        p2 = emit_conv2_chunk(q, c2ps, h2sb, mvps, acc)
                        pending_act.append((q, p2))
                    pre2()
                    while pending_mv:
                        emit_mv(*pending_mv.pop(0), mvps, acc)
                    # line A output: a^T = LeakyReLU(h^T)
                    nc.scalar.activation(aT[:], hT[:], AF.Prelu, alpha=ALPHA)
                    mva = mvps.tile([1, BC], F32, tag="mv")
                    nc.tensor.matmul(mva[:], wda[:, 0:1], aT[:, 0:128],
                                     start=True, stop=False)
                    nc.tensor.matmul(mva[:], wda[:, 1:2], aT[:, 128:256],
                                     start=False, stop=True)
                    nc.vector.tensor_add(acc[:], acc[:], mva[:])
                    out_sb = accp.tile([1, BC], F32)
                    nc.scalar.add(out_sb[:], acc[:], bds[0:1, 0:1])
                    nc.sync.dma_start(y_d, out_sb[:])
            if DEBUG_DUMP:
                nc.sync.dma_start(dbg_x2T, x2T[:])
                nc.sync.dma_start(dbg_aT, aT[:])
                nc.sync.dma_start(dbg_h1T, h1T[:])

    nc.compile()
    return nc


def TileCtx(nc):
    return tile.TileContext(nc)


def _prep_weights(Wx, Wh, b_lstm, k1, b1, k2, b2, Wd, bd):
    """Host-side weight preprocessing (gate perm, even/odd packing, casts)."""
    # gate order i,f,g,o -> f,i,o,g
    perm = np.concatenate(
        [np.arange(256, 512), np.arange(0, 256),
         np.arange(768, 1024), np.arange(512, 768)]
    )
    Wxp = Wx[:, perm].astype(np.float32)       # [51, 1024]
    Whp = Wh[:, perm].astype(np.float32)       # [256, 1024]
    bp = b_lstm[perm].astype(np.float32)       # [1024]

    wxe = np.zeros((103, 1024), np.float32)
    wxo = np.zeros((103, 1024), np.float32)
    wxe[0:51] = Wxp
    wxo[51:102] = Wxp
    wxe[102] = bp
    wxo[102] = bp
    wxe = _dt(wxe.reshape(103, 8, 128), X_BF16)
    wxo = _dt(wxo.reshape(103, 8, 128), X_BF16)

    whT = _dt(
        np.ascontiguousarray(
            Whp.reshape(2, 128, 8, 128).transpose(1, 0, 2, 3)
        ),
        WH_BF16,
    )  # [128, 2, 8, 128]: whT[p, c, j, m] = Whp[c*128+p, j*128+m]

    k1p = np.zeros((102, 3, 64), np.float32)
    k1p[0:51, 0] = k1[0]
    k1p[51:102, 0] = k1[1]
    k1p[0:51, 1] = k1[2]
    k1p[51:102, 1] = k1[3]
    k1p[0:51, 2] = k1[4]
    k1p = k1p.astype(ml_dtypes.bfloat16)

    k2p = np.zeros((128, 3, 128), np.float32)
    k2p[0:64, 0] = k2[0]
    k2p[64:128, 0] = k2[1]
    k2p[0:64, 1] = k2[2]
    k2p[64:128, 1] = k2[3]
    k2p[0:64, 2] = k2[4]
    k2p = k2p.astype(ml_dtypes.bfloat16)

    Wd = Wd.astype(np.float32)
    wda = Wd[0:256, 0].reshape(2, 128).T.copy()          # [128, 2]
    wdb = Wd[256:, 0].reshape(128, 128).T.copy()         # [c2, to2]

    return dict(
        wxe=np.ascontiguousarray(wxe),
        wxo=np.ascontiguousarray(wxo),
        whT=np.ascontiguousarray(whT),
        k1p=np.ascontiguousarray(k1p),
        k2p=np.ascontiguousarray(k2p),
        wdb=np.ascontiguousarray(wdb),
        wda=np.ascontiguousarray(wda),
        b1=b1.astype(np.float32).reshape(64, 1),
        b2=b2.astype(np.float32).reshape(128, 1),
        bd=bd.astype(np.float32).reshape(1, 1),
    )


def _prep_x2t(xc, t_steps):
    """Per-core x -> transposed even/odd-packed layout [103, nt2, BC].

    Rows 0:51 = features of even t, 51:102 = odd t, row 102 = ones (the
    LSTM bias row folded into the input projection).
    """
    bc = xc.shape[0]
    nt2 = (t_steps + 1) // 2
    x2 = np.empty((103, nt2, bc), np.float32)
    x2[0:51] = xc[:, 0::2, :].transpose(2, 1, 0)
    x2[51:102] = xc[:, 1::2, :].transpose(2, 1, 0)
    x2[102] = 1.0
    return np.ascontiguousarray(_dt(x2, X_BF16))


def _get_nc(t_steps=T):
    key = (t_steps, WH_BF16, X_BF16)
    if key not in _NC_CACHE:
        _NC_CACHE[key] = build_nc(t_steps)
    return _NC_CACHE[key]


def run(inputs, t_steps=T, trace=False):
    """Run the SPMD kernel; returns ([B,1] output, BassKernelResults)."""
    x = np.asarray(inputs["x"], np.float32)
    weights = _prep_weights(
        np.asarray(inputs["Wx"]), np.asarray(inputs["Wh"]),
        np.asarray(inputs["b_lstm"]), np.asarray(inputs["k1"]),
        np.asarray(inputs["b1"]), np.asarray(inputs["k2"]),
        np.asarray(inputs["b2"]), np.asarray(inputs["Wd"]),
        np.asarray(inputs["bd"]),
    )
    nc = _get_nc(t_steps)
    in_maps = []
    for i in range(NCORES):
        m = dict(weights)
        m["x2t"] = _prep_x2t(x[i * BC : (i + 1) * BC, :t_steps], t_steps)
        in_maps.append(m)
    res = run_bass_kernel_spmd(
        nc, in_maps, core_ids=list(range(NCORES)), trace=trace
    )
    out = np.empty((B, 1), np.float32)
    for i in range(NCORES):
        out[i * BC : (i + 1) * BC, 0] = res.results[i]["y"][0]
    return out, res


def kernel(**inputs):
    out, _ = run(inputs)
    return out


def bench(inputs, iters=20, t_steps=T):
    """Steady-state timing: device-resident inputs, repeated execution.

    Returns (per-iter seconds list, output of last iter as [B,1]).
    """
    import time
    import jax
    from jax.sharding import Mesh, PartitionSpec
    from jax.experimental.shard_map import shard_map
    from concourse import bass2jax
    from concourse import mybir as _mybir

    x = np.asarray(inputs["x"], np.float32)
    weights = _prep_weights(
        np.asarray(inputs["Wx"]), np.asarray(inputs["Wh"]),
        np.asarray(inputs["b_lstm"]), np.asarray(inputs["k1"]),
        np.asarray(inputs["b1"]), np.asarray(inputs["k2"]),
        np.asarray(inputs["b2"]), np.asarray(inputs["Wd"]),
        np.asarray(inputs["bd"]),
    )
    nc = _get_nc(t_steps)
    in_maps = []
    for i in range(NCORES):
        m = dict(weights)
        m["x2t"] = _prep_x2t(x[i * BC : (i + 1) * BC, :t_steps], t_steps)
        in_maps.append(m)

    bass2jax.install_neuronx_cc_hook()
    # replicate run_bass_via_pjrt's sharded path, but keep the jitted fn
    in_names, out_names, out_avals, zero_outs = [], [], [], []
    partition_name = (
        nc.partition_id_tensor.name if nc.partition_id_tensor else None
    )
    for alloc in nc.m.functions[0].allocations:
        if not isinstance(alloc, _mybir.MemoryLocationSet):
            continue
        name = alloc.memorylocations[0].name
        if alloc.kind == "ExternalInput":
            if name != partition_name:
                in_names.append(name)
        elif alloc.kind == "ExternalOutput":
            out_names.append(name)
            shape = tuple(alloc.tensor_shape)
            dtype = _mybir.dt.np(alloc.dtype)
            out_avals.append(jax.core.ShapedArray(shape, dtype))
            zero_outs.append(np.zeros(shape, dtype))
    n_params = len(in_names)
    n_outs = len(out_avals)
    in_names_all = in_names + out_names
    if partition_name is not None:
        in_names_all = in_names_all + [partition_name]
    donate = tuple(range(n_params, n_params + n_outs))

    def _body(*args):
        operands = list(args)
        if partition_name is not None:
            operands.append(bass2jax.partition_id_tensor())
        outs = bass2jax._bass_exec_p.bind(
            *operands,
            out_avals=tuple(out_avals),
            in_names=tuple(in_names_all),
            out_names=tuple(out_names),
            lowering_input_output_aliases=(),
            sim_require_finite=True,
            sim_require_nnan=True,
            nc=nc,
        )
        return tuple(outs)

    devices = jax.devices()[:NCORES]
    mesh = Mesh(np.asarray(devices), ("core",))
    in_specs = (PartitionSpec("core"),) * (n_params + n_outs)
    out_specs = (PartitionSpec("core"),) * len(out_names)
    sharded = jax.jit(
        shard_map(_body, mesh=mesh, in_specs=in_specs, out_specs=out_specs,
                  check_rep=False),
        donate_argnums=donate, keep_unused=True,
    )
    concat_in = [
        np.concatenate([np.asarray(in_maps[c][nm]) for c in range(NCORES)], axis=0)
        for nm in in_names
    ]
    sh = jax.sharding.NamedSharding(mesh, PartitionSpec("core"))
    dev_in = [jax.device_put(a, sh) for a in concat_in]

    def one_iter():
        zeros = [
            np.zeros((NCORES * z.shape[0], *z.shape[1:]), z.dtype)
            for z in zero_outs
        ]
        outs = sharded(*dev_in, *zeros)
        jax.block_until_ready(outs)
        return outs

    outs = one_iter()  # warmup/compile
    times = []
    for _ in range(iters):
        t0 = time.perf_counter()
        outs = one_iter()
        times.append(time.perf_counter() - t0)
    yi = out_names.index("y")
    yarr = np.asarray(outs[yi]).reshape(NCORES, 1, BC)
    out = np.empty((B, 1), np.float32)
    for i in range(NCORES):
        out[i * BC : (i + 1) * BC, 0] = yarr[i, 0]
    return times, out

